# revision 2
# baseline (speedup 1.0000x reference)
"""Soft k-means (DCN vq_codebook) on 8 Trainium2 NeuronCores — moment-space
formulation.

Math (reference): 10 iterations of
    d    = ||x||^2 + ||c||^2 - 2 X C^T                    [N, K]
    dn   = (d - dmin) / (dmax - dmin)
    soft = exp(-gamma * dn);  sp = soft / rowsum(soft)
    C    = (sp^T X) / colsum(sp)                          [K, D]

Validated transformations (bf16-faithful numpy sim, rel err ~9e-6 vs the
exact 10-iteration reference, tolerance 2e-3):
  * Row factors (||x||^2, dmin) cancel in the row normalization, so
    sp row-derives from exp(a*(cc_k - 2 x.c_k)) with a = -gamma/R and a
    frozen R from the Cauchy-Schwarz bound (output insensitive to R +-2x).
  * |a*delta| <= gamma = 0.01, so exp linearizes: soft ~= 1 + a*delta.
    The whole [N, K] pipeline then collapses into 65x65 moments:
        r_n  = K + a*(CC - 2 x_n.csum)          (csum = sum_k c_k)
        S    = [X*ir | ir]^T [X | 1]            (ir = 1/r)   [65, 65]
        W^T  = s*(1+a*cc)^T - 2a*(M C^T),  mass = z*(1+a*cc) - 2a*(s.C^T)
    with M = S[:64,:64], s = S[64,:64], z = S[64,64]; C_new = W/mass.
  * ir = 1/K + w splits the matmul into a one-time X^T X part (Sb0) and a
    per-iteration small-signal part (full relative precision in bf16).
  * The sum-X row/col of Sb0 must come from an f32 path (summing 131072
    bf16 roundings biases the dominant common mode by ~0.1%).
  * 2 iterations reproduce the reference's 10 (contraction ~3e3 per iter).
  * Data-parallel over N: per-core [65, 65] partial moments, AllReduce-add
    per iteration; plus one tiny AllReduce-max for the frozen R statistic.
"""

import os
import sys

sys.path.insert(0, "/opt/trn_rl_repo")

import numpy as np

import concourse.bacc as bacc
import concourse.bass as bass
import concourse.mybir as mybir
import concourse.tile as tile
from concourse import bass_utils

F32 = mybir.dt.float32
BF16 = mybir.dt.bfloat16
AF = mybir.ActivationFunctionType
ALU = mybir.AluOpType
AX = mybir.AxisListType

NCORES = 8
N, D, K = 131072, 64, 1024
NL = N // NCORES          # rows per core (16384)
NT = NL // 128            # 128-row tiles per core (128)
E1 = D + 1                # 65: [x | 1] / [x*w | w] widths
NTXX = 16                 # tiles sampled for the max||x||^2 statistic
ITERS = int(os.environ.get("VQ_ITERS", "2"))
SKIP_AR = bool(int(os.environ.get("VQ_SKIP_AR", "0")))  # timing experiments only
GAMMA = 0.01
INVK = 1.0 / K


def _build_module():
    nc = bacc.Bacc("TRN2", target_bir_lowering=False, debug=False,
                   enable_asserts=False, num_devices=NCORES)

    in_Xn = nc.dram_tensor("in_xn", [128, NT * D], F32, kind="ExternalInput").ap()
    in_CT = nc.dram_tensor("in_ct", [D, K], F32, kind="ExternalInput").ap()
    in_id = nc.dram_tensor("in_id", [128, 128], F32, kind="ExternalInput").ap()
    # identical row-sample on every core: replaces the max-AllReduce for the
    # frozen-R statistic (every core derives the same a locally)
    in_Xs = nc.dram_tensor("in_xs", [128, NTXX * D], F32, kind="ExternalInput").ap()
    out_CT = nc.dram_tensor("out_ct", [D, K], F32, kind="ExternalOutput").ap()

    with tile.TileContext(nc) as tc:
        with tc.tile_pool(name="per", bufs=1) as per, \
             tc.tile_pool(name="psa", bufs=1, space="PSUM") as psa, \
             tc.tile_pool(name="psb", bufs=1, space="PSUM") as psb, \
             tc.tile_pool(name="psw", bufs=1, space="PSUM") as psw, \
             tc.tile_pool(name="psm", bufs=1, space="PSUM") as psm, \
             tc.tile_pool(name="pst", bufs=1, space="PSUM") as pst, \
             tc.tile_pool(name="psu", bufs=1, space="PSUM") as psu, \
             tc.tile_pool(name="dram", bufs=1, space="DRAM") as dram:

            # ---------------- persistent tiles ----------------
            Xn = per.tile([128, NT * D], F32, tag="xn")
            Xb = per.tile([128, NT * E1], BF16, tag="xb")     # [x | 1] tiles
            Xw = per.tile([128, NT * E1], BF16, tag="xw")     # [x*w | w] tiles
            Xs = per.tile([128, NTXX * D], F32, tag="xs")
            CT = per.tile([D, K], F32, tag="ct")
            CTb = per.tile([D, K], BF16, tag="ctb")
            CTsq = per.tile([D, K], BF16, tag="ctsq")
            rhs2 = per.tile([1, K], F32, tag="rhs2")
            mrow = per.tile([1, K], F32, tag="mrow")
            invm = per.tile([1, K], F32, tag="invm")
            WM_sb = per.tile([E1, K], F32, tag="wmsb")        # [W^T; mass]
            lhsT1 = per.tile([D, E1], BF16, tag="lhst1")
            Sb0K = per.tile([E1, E1], F32, tag="sb0k")        # Sb0/K, strips fixed
            S_sb = per.tile([E1, E1], F32, tag="s_sb")
            S2 = per.tile([E1, E1], F32, tag="s2")
            srow = per.tile([1, E1], F32, tag="srow")         # S2 row 64 at p0
            ident = per.tile([128, 128], F32, tag="ident")
            ones128 = per.tile([1, 128], F32, tag="ones128")
            onesf = per.tile([128, 1], F32, tag="onesf")
            ones64b = per.tile([D, 1], BF16, tag="ones64b")
            u_sb = per.tile([128, NT], F32, tag="u")
            r_sb = per.tile([128, NT], F32, tag="r")
            ir_sb = per.tile([128, NT], F32, tag="ir")
            w_sb = per.tile([128, NT], F32, tag="w")
            xx = per.tile([128, NTXX], F32, tag="xx")
            mxp = per.tile([128, 1], F32, tag="mxp")
            row128 = per.tile([1, 128], F32, tag="row128")
            csum = per.tile([D, 1], F32, tag="csum")
            csr = per.tile([1, D], F32, tag="csr")
            csB = per.tile([128, D], F32, tag="csb")
            sumX_c = per.tile([D, 1], F32, tag="sumxc")       # (sum X)/K col
            sumX_r = per.tile([1, D], F32, tag="sumxr")       # (sum X)/K row
            t3 = per.tile([128, D], F32, tag="t3")
            sc = per.tile([1, 12], F32, tag="sc")
            a_col = per.tile([128, 1], F32, tag="acol")
            a2_col = per.tile([128, 1], F32, tag="a2col")
            b_col = per.tile([128, 1], F32, tag="bcol")

            psA = psa.tile([E1, E1], F32, tag="psa")          # Sb0 (kept)
            psB = psb.tile([E1, E1], F32, tag="psb")          # S_dev per iter
            psW = psw.tile([E1, K], F32, tag="psw")           # [W^T; mass]
            psM = psm.tile([D, K], F32, tag="psm")            # cc row / invmB
            psT = pst.tile([128, 128], F32, tag="pst")        # transposes
            psU = psu.tile([128, E1], F32, tag="psu")         # csumB | b_col

            dS_i = dram.tile([E1, E1], F32, tag="ds_i")
            dS_o = dram.tile([E1, E1], F32, tag="ds_o")

            # ---------------- input DMAs + constants ----------------
            nc.sync.dma_start(ident[:], in_id)
            nc.sync.dma_start(CT[:], in_CT)
            nc.sync.dma_start(Xs[:], in_Xs)
            nc.sync.dma_start(Xn[:], in_Xn)
            nc.vector.memset(ones128[:], 1.0)
            nc.vector.memset(onesf[:], 1.0)
            nc.vector.memset(ones64b[:], 1.0)

            xn3 = Xn[:].rearrange("p (t e) -> p t e", e=D)
            xb3 = Xb[:].rearrange("p (t e) -> p t e", e=E1)
            xw3 = Xw[:].rearrange("p (t e) -> p t e", e=E1)
            w3 = w_sb[:].rearrange("p (t o) -> p t o", o=1)

            def cc_csum_of_CT():
                """cc row (psM[0:1,:]), csum col, csumB (psU[:,0:D]), CC."""
                nc.scalar.activation(CTsq[:], CT[:], AF.Square)
                nc.tensor.matmul(psM[0:1, 0:512], lhsT=ones64b[:],
                                 rhs=CTsq[:, 0:512], start=True, stop=True)
                nc.tensor.matmul(psM[0:1, 512:1024], lhsT=ones64b[:],
                                 rhs=CTsq[:, 512:1024], start=True, stop=True)
                nc.vector.tensor_reduce(csum[:], CT[:], axis=AX.X, op=ALU.add)
                nc.tensor.transpose(psT[0:1, 0:D], csum[:], ident[0:D, 0:D])
                nc.vector.tensor_copy(csr[:], psT[0:1, 0:D])
                nc.tensor.matmul(psU[:, 0:D], lhsT=ones128[:], rhs=csr[:],
                                 start=True, stop=True)
                nc.scalar.copy(csB[:], psU[:, 0:D])
                nc.vector.tensor_reduce(sc[:, 0:1], psM[0:1, 0:K],
                                        axis=AX.X, op=ALU.add)

            def b_broadcast():
                """b = a*CC + K broadcast to b_col [128,1]."""
                nc.vector.tensor_scalar(sc[:, 1:2], sc[:, 0:1], sc[:, 8:9],
                                        float(K), op0=ALU.mult, op1=ALU.add)
                nc.tensor.matmul(psU[:, D:E1], lhsT=ones128[:], rhs=sc[:, 1:2],
                                 start=True, stop=True)
                nc.vector.tensor_copy(b_col[:], psU[:, D:E1])

            def u_of_csum():
                """u[p,t] = x . csum via bf16 product sink + f32 reduce."""
                csb3 = csB[:].rearrange("p (o e) -> p o e", o=1)
                cB, xB = bass.broadcast_tensor_aps(csb3, xn3)
                nc.gpsimd.tensor_mul(xw3[:, :, 0:D], xB, cB)
                nc.vector.tensor_reduce(u_sb[:], xw3[:, :, 0:D],
                                        axis=AX.X, op=ALU.add)

            def post_update():
                """S2 [65,65] (+srow at p0) + ccsb + CT -> CT_new (in place)."""
                nc.scalar.activation(lhsT1[:], S2[0:D, 0:E1], AF.Copy,
                                     scale=a2_col[0:D, :])
                nc.scalar.activation(CTb[:], CT[:], AF.Copy)
                nc.scalar.activation(rhs2[:], psM[0:1, 0:K], AF.Identity,
                                     bias=1.0, scale=a_col[0:1, :])
                nc.tensor.matmul(psW[:, 0:512], lhsT=lhsT1[:],
                                 rhs=CTb[:, 0:512], start=True, stop=False)
                nc.tensor.matmul(psW[:, 512:1024], lhsT=lhsT1[:],
                                 rhs=CTb[:, 512:1024], start=True, stop=False)
                nc.tensor.matmul(psW[:, 0:512], lhsT=srow[:],
                                 rhs=rhs2[:, 0:512], start=False, stop=True)
                nc.tensor.matmul(psW[:, 512:1024], lhsT=srow[:],
                                 rhs=rhs2[:, 512:1024], start=False, stop=True)
                nc.scalar.copy(WM_sb[:], psW[:])
                nc.sync.dma_start(mrow[:], WM_sb[D:E1, 0:K])
                nc.vector.reciprocal(invm[:], mrow[:])
                nc.tensor.matmul(psM[:, 0:512], lhsT=ones128[:, 0:D],
                                 rhs=invm[:, 0:512], start=True, stop=True)
                nc.tensor.matmul(psM[:, 512:1024], lhsT=ones128[:, 0:D],
                                 rhs=invm[:, 512:1024], start=True, stop=True)
                nc.vector.tensor_mul(CT[:], WM_sb[0:D, 0:K], psM[:, 0:K])

            # ---------------- setup ----------------
            # iter-1 prologue on C0 (independent of X / a)
            cc_csum_of_CT()

            # mx statistic from the replicated row sample (identical on every
            # core -> identical a, no collective needed)
            xs3 = Xs[:].rearrange("p (t e) -> p t e", e=D)
            nc.gpsimd.tensor_mul(xw3[:, 0:NTXX, 0:D], xs3, xs3)
            nc.vector.tensor_reduce(xx[:], xw3[:, 0:NTXX, 0:D],
                                    axis=AX.X, op=ALU.add)
            nc.vector.tensor_reduce(mxp[:], xx[:], axis=AX.X, op=ALU.max)
            nc.tensor.transpose(psT[0:1, 0:128], mxp[:], ident[:])
            nc.vector.tensor_copy(row128[:], psT[0:1, 0:128])
            nc.vector.tensor_reduce(sc[:, 2:3], row128[:], axis=AX.X, op=ALU.max)

            # u1 (needs only csumB + X)
            u_of_csum()

            # Xb = [x | 1] bf16; Sb0 = sum_t Xb_t^T Xb_t
            nc.scalar.activation(xb3[:, :, 0:D], xn3, AF.Copy)
            nc.vector.memset(xb3[:, :, D:E1], 1.0)
            for t in range(NT):
                bt = Xb[:, t * E1:(t + 1) * E1]
                nc.tensor.matmul(psA[:], lhsT=bt, rhs=bt,
                                 start=(t == 0), stop=(t == NT - 1))

            # sumX/K in f32: strided reduce + partition-sum + transpose
            nc.vector.tensor_reduce(t3[:], Xn[:].rearrange("p (t e) -> p e t", e=D),
                                    axis=AX.X, op=ALU.add)
            nc.tensor.matmul(psT[0:D, 1:2], lhsT=t3[:], rhs=onesf[:],
                             start=True, stop=True)
            nc.vector.tensor_scalar_mul(sumX_c[:], psT[0:D, 1:2], INVK)
            nc.tensor.transpose(psT[0:1, 0:D], sumX_c[:], ident[0:D, 0:D])
            nc.vector.tensor_copy(sumX_r[:], psT[0:1, 0:D])

            # Sb0K = Sb0/K with f32 sum-X strips patched in
            nc.vector.tensor_scalar_mul(Sb0K[:], psA[:], INVK)
            nc.vector.tensor_copy(Sb0K[0:D, D:E1], sumX_c[:])
            nc.sync.dma_start(Sb0K[D:E1, 0:D], sumX_r[:])

            # a = -gamma / (mx + mc + 2 sqrt(mx*mc)); broadcasts
            nc.vector.tensor_reduce(sc[:, 3:4], psM[0:1, 0:K],
                                    axis=AX.X, op=ALU.max)
            nc.vector.tensor_mul(sc[:, 4:5], sc[:, 2:3], sc[:, 3:4])
            nc.scalar.activation(sc[:, 5:6], sc[:, 4:5], AF.Sqrt)
            nc.vector.tensor_add(sc[:, 6:7], sc[:, 2:3], sc[:, 3:4])
            nc.vector.tensor_scalar_mul(sc[:, 7:8], sc[:, 5:6], 2.0)
            nc.vector.tensor_add(sc[:, 9:10], sc[:, 6:7], sc[:, 7:8])
            nc.vector.reciprocal(sc[:, 10:11], sc[:, 9:10])
            nc.vector.tensor_scalar_mul(sc[:, 8:9], sc[:, 10:11], -GAMMA)
            nc.tensor.matmul(psT[0:128, 2:3], lhsT=ones128[:], rhs=sc[:, 8:9],
                             start=True, stop=True)
            nc.vector.tensor_copy(a_col[:], psT[0:128, 2:3])
            nc.vector.tensor_scalar_mul(a2_col[:], a_col[:], -2.0)

            # ---------------- iterations ----------------
            for it in range(ITERS):
                if it > 0:
                    cc_csum_of_CT()
                    u_of_csum()
                b_broadcast()
                nc.vector.tensor_scalar(r_sb[:], u_sb[:], a2_col[:], b_col[:],
                                        op0=ALU.mult, op1=ALU.add)
                nc.vector.reciprocal(ir_sb[:], r_sb[:])
                nc.vector.tensor_scalar_add(w_sb[:], ir_sb[:], -INVK)
                wB, xB2 = bass.broadcast_tensor_aps(w3, xn3)
                nc.gpsimd.tensor_mul(xw3[:, :, 0:D], xB2, wB)
                nc.gpsimd.tensor_copy(xw3[:, :, D:E1], w3)
                for t in range(NT):
                    nc.tensor.matmul(psB[:], lhsT=Xw[:, t * E1:(t + 1) * E1],
                                     rhs=Xb[:, t * E1:(t + 1) * E1],
                                     start=(t == 0), stop=(t == NT - 1))
                # S_local = Sb0/K + S_dev (strips pre-patched in Sb0K)
                nc.vector.tensor_add(S_sb[:], Sb0K[:], psB[:])
                nc.gpsimd.dma_start(dS_i[:], S_sb[:])
                if not SKIP_AR:
                    nc.gpsimd.collective_compute("AllReduce", ALU.add,
                                                 replica_groups=[list(range(NCORES))],
                                                 ins=[dS_i.opt()], outs=[dS_o.opt()])
                    nc.gpsimd.dma_start(S2[:], dS_o[:])
                    nc.gpsimd.dma_start(srow[:], dS_o[D:E1, 0:E1])
                else:
                    nc.gpsimd.dma_start(S2[:], dS_i[:])
                    nc.gpsimd.dma_start(srow[:], dS_i[D:E1, 0:E1])
                post_update()

            nc.sync.dma_start(out_CT, CT[:])

    nc.finalize()
    return nc


_NC_CACHE = None


def _get_module():
    global _NC_CACHE
    if _NC_CACHE is None:
        _NC_CACHE = _build_module()
    return _NC_CACHE


def _marshal(X, clusters):
    X = np.ascontiguousarray(np.asarray(X, np.float32))
    C0 = np.ascontiguousarray(np.asarray(clusters, np.float32))
    ident = np.eye(128, dtype=np.float32)
    CT0 = np.ascontiguousarray(C0.T)
    # identical sample shard for the frozen-R stat (tiled like in_xn)
    xs = np.ascontiguousarray(
        X[:NTXX * 128].reshape(NTXX, 128, D).transpose(1, 0, 2)
        .reshape(128, NTXX * D))
    in_maps = []
    for c in range(NCORES):
        Xc = X[c * NL:(c + 1) * NL]
        tiles = Xc.reshape(NT, 128, D).transpose(1, 0, 2)      # [128, NT, D]
        xn = np.ascontiguousarray(tiles.reshape(128, NT * D))
        in_maps.append({"in_xn": xn, "in_ct": CT0, "in_id": ident,
                        "in_xs": xs})
    return in_maps


def kernel(X, clusters):
    nc = _get_module()
    in_maps = _marshal(X, clusters)
    trace = bool(int(os.environ.get("VQ_TRACE", "0")))
    last_err = None
    for attempt in range(2):
        try:
            res = bass_utils.run_bass_kernel_spmd(
                nc, [m.copy() for m in in_maps],
                core_ids=list(range(NCORES)), trace=trace)
            break
        except Exception as e:  # wedged device: retry once in-process
            last_err = e
            if attempt == 1:
                raise
    kernel.last_results = res
    ct = np.asarray(res.results[0]["out_ct"], np.float32)
    return np.ascontiguousarray(ct.T)
